# revision 12
# baseline (speedup 1.0000x reference)
"""Trainium2 Bass kernel for nn_AttentionConv2D (sparse_attention).

The reference module reduces (due to the faithful `pos`-never-incremented
bug in its im2col expansion) to:

    Q  = Wq x + bq                      (per pixel)
    Kb = Wk x_sh + (bk + Wk pe0)        x_sh = x shifted by (+1,+1), zero-pad
    V  = Wv x_sh + bv
    a0 = s * <Q, Kb>                    s = A**-0.5
    a_p = <x, s*Wq^T kp> + s*<bq, kp>   kp = Wk pe_p + bk,  p = 1..8
    w0 = exp(a0) / (exp(a0) + sum_p exp(a_p)) + EPS
    out = V * w0, zeroed at h=H-1 / w=W-1

Sharding: data-parallel over batch, one image (256 x 64 x 64) per core on
8 NeuronCores.  Per-core layout: channels on partitions (2 chunks of 128),
pixels (flattened h*64+w) on the free dim, processed in 8 tiles of 512
pixels.  The (+1,+1) shift is a flat offset of 65 pixels; wrap-around
columns are masked to zero via the w0 row.
"""

import os
import sys

import numpy as np

for _p in ("/opt/trn_rl_repo",):
    if _p not in sys.path:
        sys.path.append(_p)

import concourse.bass as bass
import concourse.tile as tile
from concourse import bacc, mybir
from concourse import bass_utils

F32 = mybir.dt.float32
BF16 = mybir.dt.bfloat16
AF = mybir.ActivationFunctionType
ALU = mybir.AluOpType

B, C, H, W = 8, 256, 64, 64
HW = H * W              # 4096
A = 256
NT = 8                  # pixel tiles per core
TW = HW // NT           # 512 pixels per tile
SHIFT = W + 1           # 65
XCOLS = HW + 68         # padded x columns
EPS = 1e-8
SCALE = A ** -0.5
NCORES = 8

_CACHE = {}

LAST_RESULTS = None     # BassKernelResults of the most recent run (for test.py)


def _build():
    nc = bacc.Bacc("TRN2", target_bir_lowering=False, debug=False)

    x_d = nc.dram_tensor("x", [C, XCOLS], BF16, kind="ExternalInput").ap()
    wqu_d = nc.dram_tensor("wqu", [C, A + 8], BF16, kind="ExternalInput").ap()
    wkt_d = nc.dram_tensor("wkt", [C, A], BF16, kind="ExternalInput").ap()
    wvt_d = nc.dram_tensor("wvt", [C, A], BF16, kind="ExternalInput").ap()
    bq_d = nc.dram_tensor("bq2", [A, 1], F32, kind="ExternalInput").ap()
    bk_d = nc.dram_tensor("bk2", [A, 1], F32, kind="ExternalInput").ap()
    bv_d = nc.dram_tensor("bv2", [A, 1], F32, kind="ExternalInput").ap()
    cp_d = nc.dram_tensor("cp8", [8, 1], F32, kind="ExternalInput").ap()
    out_d = nc.dram_tensor("out", [C, HW], F32, kind="ExternalOutput").ap()

    with tile.TileContext(nc) as tc:
        with (
            tc.tile_pool(name="const", bufs=1) as const,
            tc.tile_pool(name="work", bufs=2) as work,
            tc.tile_pool(name="outp", bufs=3) as outp,
            tc.tile_pool(name="psA", bufs=1, space="PSUM") as psA,
            tc.tile_pool(name="psB", bufs=1, space="PSUM") as psB,
        ):
            # ---- persistent inputs ----
            x_sb = [const.tile([128, XCOLS], BF16, name=f"x{k}", tag=f"x{k}") for k in range(2)]
            wqu_sb = [const.tile([128, A + 8], BF16, name=f"wqu{k}", tag=f"wqu{k}") for k in range(2)]
            wkt_sb = [const.tile([128, A], BF16, name=f"wkt{k}", tag=f"wkt{k}") for k in range(2)]
            wvt_sb = [const.tile([128, A], BF16, name=f"wvt{k}", tag=f"wvt{k}") for k in range(2)]
            bq_sb = [const.tile([128, 1], F32, name=f"bq{a}", tag=f"bq{a}") for a in range(2)]
            bk_sb = [const.tile([128, 1], F32, name=f"bk{a}", tag=f"bk{a}") for a in range(2)]
            bv_sb = [const.tile([128, 1], F32, name=f"bv{a}", tag=f"bv{a}") for a in range(2)]
            cp_sb = const.tile([8, 1], F32, name="cp", tag="cp")
            ones_sb = const.tile([128, 128], BF16, name="ones", tag="ones")

            for k in range(2):
                r = slice(k * 128, (k + 1) * 128)
                nc.sync.dma_start(x_sb[k][:], x_d[r, :])
                nc.sync.dma_start(wqu_sb[k][:], wqu_d[r, :])
                nc.sync.dma_start(wkt_sb[k][:], wkt_d[r, :])
                nc.sync.dma_start(wvt_sb[k][:], wvt_d[r, :])
                nc.sync.dma_start(bq_sb[k][:], bq_d[r, :])
                nc.sync.dma_start(bk_sb[k][:], bk_d[r, :])
                nc.sync.dma_start(bv_sb[k][:], bv_d[r, :])
            nc.sync.dma_start(cp_sb[:], cp_d[:])
            nc.gpsimd.memset(ones_sb[:], 1.0)

            for t in range(NT):
                p0 = t * TW
                q_ps = [psA.tile([128, TW], F32, name=f"q{a}", tag=f"q{a}") for a in range(2)]
                k_ps = [psA.tile([128, TW], F32, name=f"k{a}", tag=f"k{a}") for a in range(2)]
                v_ps = [psA.tile([128, TW], F32, name=f"v{a}", tag=f"v{a}") for a in range(2)]
                s1_ps = psB.tile([128, TW], F32, name="s1", tag="s1")  # a_rest rows 0:8
                a0_ps = psB.tile([128, TW], F32, name="a0bc", tag="s2")  # a0, bcast

                xt = [x_sb[k][:, p0:p0 + TW] for k in range(2)]
                xs = [x_sb[k][:, p0 + SHIFT:p0 + SHIFT + TW] for k in range(2)]

                # Q (and a_rest via the U columns of wqu), then K, then V
                for a in range(2):
                    for k in range(2):
                        nc.tensor.matmul(
                            q_ps[a][:],
                            wqu_sb[k][:, a * 128:(a + 1) * 128],
                            xt[k],
                            start=(k == 0), stop=(k == 1),
                        )
                for k in range(2):
                    nc.tensor.matmul(
                        s1_ps[0:8, :],
                        wqu_sb[k][:, A:A + 8],
                        xt[k],
                        start=(k == 0), stop=(k == 1),
                    )
                for a in range(2):
                    for k in range(2):
                        nc.tensor.matmul(
                            k_ps[a][:],
                            wkt_sb[k][:, a * 128:(a + 1) * 128],
                            xs[k],
                            start=(k == 0), stop=(k == 1),
                        )

                # Kb = K + bk'  (PSUM -> SBUF with per-partition bias, ACT)
                kb_sb = [work.tile([128, TW], BF16, name=f"kb{a}", tag=f"kb{a}") for a in range(2)]
                for a in range(2):
                    nc.scalar.activation(
                        kb_sb[a][:], k_ps[a][:], AF.Identity,
                        bias=bk_sb[a][:], scale=1.0,
                    )

                # V last so tile t+1's other matmuls don't stall on v_ps
                for a in range(2):
                    for k in range(2):
                        nc.tensor.matmul(
                            v_ps[a][:],
                            wvt_sb[k][:, a * 128:(a + 1) * 128],
                            xs[k],
                            start=(k == 0), stop=(k == 1),
                        )

                # prod = (Q + bq) * Kb  (DVE, fused bias)
                prod_sb = [work.tile([128, TW], BF16, name=f"pr{a}", tag=f"pr{a}") for a in range(2)]
                for a in range(2):
                    nc.vector.scalar_tensor_tensor(
                        prod_sb[a][:], q_ps[a][:], bq_sb[a][:], kb_sb[a][:],
                        ALU.add, ALU.mult,
                    )

                # a0 = colsum(prod), broadcast to all 128 partitions via
                # all-ones stationary operand (M=128 costs the same as M=1)
                for a in range(2):
                    nc.tensor.matmul(
                        a0_ps[:], ones_sb[:], prod_sb[a][:],
                        start=(a == 0), stop=(a == 1),
                    )

                # exp8 = exp(a_rest + cp), e2 = exp(s*a0) (broadcast)
                exp8_sb = work.tile([8, TW], BF16, name="exp8", tag="exp8")
                nc.scalar.activation(
                    exp8_sb[:], s1_ps[0:8, :], AF.Exp,
                    bias=cp_sb[:], scale=1.0,
                )
                e2_sb = work.tile([128, TW], BF16, name="e2", tag="e2")
                nc.scalar.activation(
                    e2_sb[:], a0_ps[:], AF.Exp,
                    bias=0.0, scale=SCALE,
                )

                # D = e2 + sum of the 8 exp rows, broadcast (reuses k0's bank,
                # which is free once Kb has been copied out)
                d_ps = psA.tile([128, TW], F32, name="dbc", tag="k0")
                nc.tensor.matmul(
                    d_ps[:], ones_sb[0:8, :], exp8_sb[:],
                    start=True, stop=False,
                )
                nc.tensor.matmul(
                    d_ps[:], ones_sb[0:1, :], e2_sb[0:1, :],
                    start=False, stop=True,
                )

                # w0 = exp(s*a0) / D + EPS, with boundary mask
                r_sb = work.tile([128, TW], F32, name="recip", tag="recip")
                nc.vector.reciprocal_approx_fast(r_sb[:], d_ps[:])
                w0_sb = work.tile([128, TW], F32, name="w0", tag="w0")
                nc.vector.tensor_mul(w0_sb[:], e2_sb[:], r_sb[:])
                w0e_sb = work.tile([128, TW], F32, name="w0e", tag="w0e")
                nc.vector.tensor_scalar_add(w0e_sb[:], w0_sb[:], EPS)
                # mask: zero w == W-1 columns (shift wrap) and, in the last
                # tile, the h == H-1 rows
                nc.gpsimd.memset(w0e_sb[:, W - 1:TW:W], 0.0)
                if t == NT - 1:
                    nc.gpsimd.memset(w0e_sb[:, TW - W:TW], 0.0)

                # out = (V + bv) * w0
                out_sb = [outp.tile([128, TW], F32, name=f"o{a}", tag=f"o{a}") for a in range(2)]
                for a in range(2):
                    nc.vector.scalar_tensor_tensor(
                        out_sb[a][:], v_ps[a][:], bv_sb[a][:], w0e_sb[:],
                        ALU.add, ALU.mult,
                    )
                    nc.sync.dma_start(
                        out_d[a * 128:(a + 1) * 128, p0:p0 + TW], out_sb[a][:]
                    )

    nc.compile()
    return nc


def _host_prep(x, Wq, bq, Wk, bk, Wv, bv):
    """Precompute per-core DRAM inputs."""
    x = np.asarray(x, np.float32)
    Wq = np.asarray(Wq, np.float32)
    bq = np.asarray(bq, np.float32)
    Wk = np.asarray(Wk, np.float32)
    bk = np.asarray(bk, np.float32)
    Wv = np.asarray(Wv, np.float32)
    bv = np.asarray(bv, np.float32)

    # positional encoding (C, 9), matching reference._pos_encoding
    pos = np.arange(9, dtype=np.float32)[:, None]
    div = np.exp(np.arange(0, C, 2, dtype=np.float32) * (-np.log(10000.0) / C))
    pe = np.zeros((9, C), np.float32)
    pe[:, 0::2] = np.sin(pos * div)
    pe[:, 1::2] = np.cos(pos * div)
    pe = pe.T  # (C, 9)

    kp = Wk @ pe[:, 1:] + bk[:, None]       # (A, 8)
    U = SCALE * (Wq.T @ kp)                 # (C, 8)
    cp = SCALE * (bq @ kp)                  # (8,)
    bk2 = bk + Wk @ pe[:, 0]                # (A,)

    import ml_dtypes
    bf16 = ml_dtypes.bfloat16

    wqu = np.ascontiguousarray(np.concatenate([Wq.T, U], axis=1)).astype(bf16)
    wkt = np.ascontiguousarray(Wk.T).astype(bf16)
    wvt = np.ascontiguousarray(Wv.T).astype(bf16)

    xp = np.zeros((B, C, XCOLS), bf16)
    xp[:, :, :HW] = x.reshape(B, C, HW).astype(bf16)

    common = {
        "wqu": wqu,
        "wkt": wkt,
        "wvt": wvt,
        "bq2": np.ascontiguousarray(bq[:, None]),
        "bk2": np.ascontiguousarray(bk2[:, None]),
        "bv2": np.ascontiguousarray(bv[:, None]),
        "cp8": np.ascontiguousarray(cp[:, None]),
    }
    return [
        {"x": np.ascontiguousarray(xp[core]), **common} for core in range(NCORES)
    ]


def kernel(x, Wq, bq, Wk, bk, Wv, bv):
    global LAST_RESULTS
    if "nc" not in _CACHE:
        _CACHE["nc"] = _build()
    nc = _CACHE["nc"]

    in_maps = _host_prep(x, Wq, bq, Wk, bk, Wv, bv)
    res = bass_utils.run_bass_kernel_spmd(
        nc, in_maps, core_ids=list(range(NCORES)),
        trace=bool(os.environ.get("KERNEL_TRACE")),
    )
    LAST_RESULTS = res
    out = np.stack([res.results[i]["out"] for i in range(NCORES)], axis=0)
    return out.reshape(B, C, H, W).astype(np.float32, copy=False)
